# revision 6
# baseline (speedup 1.0000x reference)
"""Fused DropBlock_Ske + DropBlockT_1d kernel for Trainium2 (8 NeuronCores).

The reference's coordinate-attention branch is dead code w.r.t. the output,
which reduces to

    out[n,c,t,v] = x[n,c,t,v] * mv_eff[n,v] * mk_t[n,t]

where mv_eff/mk_t are masks derived from the tiny inputs (mask_s, mask_t,
u_s, u_t, A).  Two structural facts make the memory-bound pass cheap:

  * DropBlock_Ske's adjacency propagation (M_seed @ A > eps) turns any batch
    with >=1 spatial seed into an (almost) fully dropped batch, so a large
    fraction of batches have mv_eff == 0 for ALL v -> their output is
    exactly zero and never needs to touch the device.
  * The rel-err budget (2e-2) is ~20x above fp16 staging error, so the
    surviving batches are shipped to HBM as fp16 in both directions,
    halving the DMA traffic of the masked multiply.

The device kernel processes the K surviving batches as K*256 channel-rows
of T*V=3200 fp16 values, split perfectly evenly across the 8 cores
(K*32 rows each, any K): per core that is `n_full` [128,3200] tiles plus
one [part,3200] partial tile.  Each tile's per-partition mask rows
(mt:128, mv:25 fp16) ride along in tile 0's load; the device builds
comb[p,t,v] = mt[p,t]*mv[p,v] and multiplies in place, so the DropBlock
masking itself happens on-device.  Host work is only sharding/staging:
batch selection, fp16 conversion, and scattering results into the zeroed
full-size output.
"""

import numpy as np

NM, C, T, V = 64, 256, 128, 25
N_CORES = 8
TV = T * V                   # 3200
P = 128                      # SBUF partitions
MROW = T + V                 # mask row per tile per partition: mt(128)+mv(25)

KEEP_PROB = 0.9
BLOCK_SIZE = 7

# Set by test harness only: trace the run and stash results for profiling.
TRACE = False
LAST_RESULT = None

_BASS = {}


def _compute_masks(A, mask_s, mask_t, u_s, u_t):
    """Replicates the reference's mask math in float32 numpy.

    Returns mv_eff (NM, V) = mk_s * combined_scale and mk_t (NM, T)."""
    f32 = np.float32
    A = np.asarray(A, f32)
    mask_s = np.asarray(mask_s, f32)
    mask_t = np.asarray(mask_t, f32)
    u_s = np.asarray(u_s, f32)
    u_t = np.asarray(u_t, f32).reshape(NM, T)

    # ---- DropBlock_Ske ----
    gamma_s = f32((1.0 - KEEP_PROB) / (1.0 + 1.92))
    ms = mask_s / mask_s.sum() * f32(mask_s.size)
    p_s = np.minimum(ms * gamma_s, f32(1.0))
    m_seed = (u_s < p_s).astype(f32)
    m = ((m_seed @ A) > f32(0.001)).astype(f32)
    mk_s = f32(1.0) - m                                   # (NM, V), 0/1
    scale_s = float(NM * V) / max(float(mk_s.sum()), 1.0)

    # ---- DropBlockT_1d ----
    gamma_t = f32((1.0 - KEEP_PROB) / BLOCK_SIZE)
    mt = mask_t / mask_t.sum() * f32(mask_t.size)
    p_t = np.minimum(mt * gamma_t, f32(1.0))
    m_t = (u_t < p_t).astype(f32)                         # (NM, T), 0/1
    pad = BLOCK_SIZE // 2
    mp = np.pad(m_t, ((0, 0), (pad, pad)), constant_values=0.0)
    msum = m_t.copy()
    for i in range(BLOCK_SIZE):
        np.maximum(msum, mp[:, i:i + T], out=msum)
    mk_t = f32(1.0) - msum                                # (NM, T), 0/1
    numel = float(NM * C * T * V)
    scale_t = numel / max(float(mk_t.sum()) * (C * V), 1.0)

    mv_eff = mk_s * f32(scale_s * scale_t)
    return mv_eff.astype(f32), mk_t.astype(f32)


def _build_bass(n_full, part):
    """Device program for one core: n_full [128,3200] fp16 tiles plus an
    optional [part,3200] tile; tile i multiplied in place by
    comb_i[p,t,v] = mt_i[p,t] * mv_i[p,v] from mask columns riding in
    tile 0's load, then stored.  One DMA per tile each way."""
    import concourse.bass as bass
    import concourse.mybir as mybir
    from concourse.tile import TileContext, add_dep_helper

    f16 = mybir.dt.float16
    ntiles = n_full + (1 if part else 0)
    rows = [P] * n_full + ([part] if part else [])
    MOFF = TV                                # mask cols start in xm/t0
    W0 = TV + ntiles * MROW                  # tile-0 row width incl masks

    nc = bass.Bass()
    xm = nc.dram_tensor("xm", [P, W0], f16, kind="ExternalInput")
    if ntiles > 1:
        xs = nc.dram_tensor("xs", [(n_full - 1) * P + (part or 0), TV], f16,
                            kind="ExternalInput")
    out = nc.dram_tensor("out", [n_full * P + (part or 0), TV], f16,
                         kind="ExternalOutput")

    with TileContext(nc) as tc:
        with tc.tile_pool(name="t0", bufs=1) as t0pool, \
             tc.tile_pool(name="work", bufs=max(1, min(ntiles - 1, 10))) as pool, \
             tc.tile_pool(name="comb", bufs=min(ntiles, 7)) as combpool, \
             tc.tile_pool(name="scratch", bufs=ntiles) as spool, \
             tc.tile_pool(name="pscr", bufs=ntiles) as ppool:
            loads, stores, applies, pcars = [], [], [], []
            t0 = None
            for i in range(ntiles):
                r = rows[i]
                if i == 0:
                    t0 = t0pool.tile([P, W0], f16, tag="t0")
                    t = t0
                    ld = nc.sync.dma_start(t[:, :], xm[:, :])
                else:
                    t = pool.tile([P, TV], f16)
                    ld = nc.sync.dma_start(
                        t[0:r, :], xs[(i - 1) * P:(i - 1) * P + r, :])

                # comb[p, t, v] = mt_i[p, t] * mv_i[p, v]
                comb = combpool.tile([P, TV], f16)
                mc = MOFF + i * MROW
                mt_b = t0[0:r, mc:mc + T] \
                    .unsqueeze(2).broadcast_to([r, T, V])
                mv_b = t0[0:r, mc + T:mc + MROW] \
                    .unsqueeze(1).broadcast_to([r, T, V])
                comb3 = comb[0:r, :].rearrange("p (t v) -> p t v", v=V)
                cb = nc.vector.tensor_tensor(out=comb3, in0=mt_b, in1=mv_b,
                                             op=mybir.AluOpType.mult)

                # read-carrier: sole absorber of the RAW wait on the load;
                # the apply's scratch read then folds the carrier + comb
                # deps into a single DVE self-wait.
                scratch = spool.tile([P, 1], mybir.dt.float32)
                tcar = nc.vector.tensor_tensor(
                    out=scratch[0:r, :], in0=t[0:r, 1:2], in1=t[0:r, 1:2],
                    op=mybir.AluOpType.mult)
                ap = nc.vector.scalar_tensor_tensor(
                    out=t[0:r, 0:TV], in0=t[0:r, 0:TV],
                    scalar=scratch[0:r, 0:1], in1=comb[0:r, :],
                    op0=mybir.AluOpType.bypass, op1=mybir.AluOpType.mult)

                # pool-ring lane absorber: a write-only no-op with a forced
                # sync dep on the load; it carries the load-lane wait so the
                # store (whose writer list still includes the load) needs
                # only its DVE wait.
                pscr = ppool.tile([1, 1], mybir.dt.float32)
                pcar = nc.gpsimd.memset(pscr[0:1, 0:1], 0.0)
                add_dep_helper(pcar.ins, ld.ins, sync=True,
                               reason="ring lane absorber")
                st = nc.gpsimd.dma_start(
                    out[i * P:i * P + r, :], t[0:r, 0:TV])

                # --- no-sync scheduler edges (ordering only, no waits) ---
                ns = lambda a, b: add_dep_helper(a.ins, b.ins, sync=False,
                                                 reason="tick ordering")
                ns(st, pcar)
                ns(tcar, cb)                     # comb tick < carrier tick
                ns(ap, tcar)
                if i >= 1:
                    ns(tcar, applies[-1])        # keep DVE ticks monotone
                    ns(cb, applies[-1])
                    ns(ld, loads[-1])
                    ns(st, stores[-1])
                loads.append(ld)
                stores.append(st)
                applies.append(ap)
                pcars.append(pcar)
            # Tail: absorb each outstanding sem into the SP sequencer's
            # observed set with a chain of 1-wait nops so the framework
            # drain never exceeds per-instruction wait capacity.
            ptail = nc.gpsimd.memset(pscr[0:1, 0:1], 0.0)
            add_dep_helper(ptail.ins, stores[-1].ins, sync=False,
                           reason="final pool op")
            prev = None
            tail_deps = list(stores) + list(loads) + [applies[-1], ptail]
            for dep in tail_deps:
                nop = nc.sync.nop()
                add_dep_helper(nop.ins, dep.ins, sync=True,
                               reason="drain pre-absorb")
                add_dep_helper(nop.ins,
                               (prev if prev is not None else loads[-1]).ins,
                               sync=False, reason="tail order")
                prev = nop
    return nc


def kernel(x, A, mask_s, mask_t, u_s, u_t, w1, b1, bn_gamma, bn_beta,
           wh, bh, ww, bw):
    global LAST_RESULT
    from concourse.bass_utils import run_bass_kernel_spmd

    f16 = np.float16
    x = np.asarray(x, np.float32)
    mv_eff, mk_t = _compute_masks(A, mask_s, mask_t, u_s, u_t)

    surv = np.where((mv_eff != 0).any(axis=1) & (mk_t != 0).any(axis=1))[0]
    K = len(surv)
    out_full = np.zeros((NM, C, T, V), np.float32)
    if K == 0:
        return out_full

    rows_pc = K * C // N_CORES               # exact for any K
    n_full, part = divmod(rows_pc, P)
    ntiles = n_full + (1 if part else 0)

    # fp16 staging of the surviving rows + per-row mask rows
    xall = x[surv].astype(f16).reshape(K * C, TV)
    b_of_row = np.arange(K * C) // C
    maskdata = np.concatenate(
        [mk_t[surv][b_of_row], mv_eff[surv][b_of_row]], axis=1).astype(f16)

    in_maps = []
    for k in range(N_CORES):
        sl = slice(k * rows_pc, (k + 1) * rows_pc)
        rows_k = xall[sl]
        mpad = np.zeros((ntiles * P, MROW), f16)
        mpad[:rows_pc] = maskdata[sl]
        xmk = np.zeros((P, TV + ntiles * MROW), f16)
        xmk[:min(P, rows_pc), :TV] = rows_k[:P]
        xmk[:, TV:] = mpad.reshape(ntiles, P, MROW) \
            .transpose(1, 0, 2).reshape(P, ntiles * MROW)
        im = {"xm": xmk}
        if ntiles > 1:
            im["xs"] = rows_k[P:]
        in_maps.append(im)

    key = (n_full, part)
    if key not in _BASS:
        _BASS[key] = _build_bass(n_full, part)

    res = run_bass_kernel_spmd(_BASS[key], in_maps, list(range(N_CORES)),
                               trace=TRACE)
    LAST_RESULT = res

    outall = np.concatenate([res.results[k]["out"] for k in range(N_CORES)])
    out_full[surv] = outall.reshape(K, C, T, V).astype(np.float32)
    return out_full


# revision 10
# speedup vs baseline: 1.5855x; 1.5855x over previous
"""Fused DropBlock_Ske + DropBlockT_1d kernel for Trainium2 (8 NeuronCores).

The reference's coordinate-attention branch is dead code w.r.t. the output,
which reduces to

    out[n,c,t,v] = x[n,c,t,v] * mv_eff[n,v] * mk_t[n,t]

where mv_eff/mk_t are 0/s masks derived from the tiny inputs (mask_s,
mask_t, u_s, u_t, A).  Structural facts exploited:

  * DropBlock_Ske's adjacency propagation (M_seed @ A > eps) makes any
    batch with >=1 spatial seed (almost always) fully dropped, so a large
    fraction of batches is exactly zero and never touches the device.
  * Surviving batches keep all (or nearly all) joints, so the per-element
    multiplier is s * mk_t[n,t]: a function of (n,t) only.  Staged in a
    [t-partitions, (c,v)-cols] layout, it is a per-PARTITION scalar, so
    the whole masked multiply is one DVE tensor_scalar op per tile (4x
    perf mode) with the mask values shipped as data - which also keeps
    the single SPMD program valid for every core.
  * The rel-err budget (2e-2) is ~60x above fp16 staging error, so the
    surviving rows travel to/from HBM as fp16, halving DMA traffic.

Work splits at t-row granularity: K surviving batches = K*T rows of
C*V=6400 fp16 values, exactly K*16 rows per core for any K (for K=25:
3 full [128,6400] tiles + one [16,6400] partial).  Host work is only
sharding/staging: mask math on tiny inputs, batch selection, the
(n,c,t,v)->(n,t,c,v) transpose, fp16 conversion, and scattering device
results into the zeroed full-size output.  (In the measure-zero case of
a surviving batch with some joints dropped, those v-columns are zeroed
during the host-side scatter.)
"""

import numpy as np

NM, C, T, V = 64, 256, 128, 25
N_CORES = 8
CV = C * V                   # 6400
P = 128                      # SBUF partitions

KEEP_PROB = 0.9
BLOCK_SIZE = 7

# Set by test harness only: trace the run and stash results for profiling.
TRACE = False
LAST_RESULT = None

_BASS = {}


def _compute_masks(A, mask_s, mask_t, u_s, u_t):
    """Replicates the reference's mask math in float32 numpy.

    Returns mv_eff (NM, V) = mk_s * combined_scale and mk_t (NM, T)."""
    f32 = np.float32
    A = np.asarray(A, f32)
    mask_s = np.asarray(mask_s, f32)
    mask_t = np.asarray(mask_t, f32)
    u_s = np.asarray(u_s, f32)
    u_t = np.asarray(u_t, f32).reshape(NM, T)

    # ---- DropBlock_Ske ----
    gamma_s = f32((1.0 - KEEP_PROB) / (1.0 + 1.92))
    ms = mask_s / mask_s.sum() * f32(mask_s.size)
    p_s = np.minimum(ms * gamma_s, f32(1.0))
    m_seed = (u_s < p_s).astype(f32)
    m = ((m_seed @ A) > f32(0.001)).astype(f32)
    mk_s = f32(1.0) - m                                   # (NM, V), 0/1
    scale_s = float(NM * V) / max(float(mk_s.sum()), 1.0)

    # ---- DropBlockT_1d ----
    gamma_t = f32((1.0 - KEEP_PROB) / BLOCK_SIZE)
    mt = mask_t / mask_t.sum() * f32(mask_t.size)
    p_t = np.minimum(mt * gamma_t, 1.0)
    m_t = (u_t < p_t).astype(f32)                         # (NM, T), 0/1
    pad = BLOCK_SIZE // 2
    mp = np.pad(m_t, ((0, 0), (pad, pad)), constant_values=0.0)
    msum = m_t.copy()
    for i in range(BLOCK_SIZE):
        np.maximum(msum, mp[:, i:i + T], out=msum)
    mk_t = f32(1.0) - msum                                # (NM, T), 0/1
    numel = float(NM * C * T * V)
    scale_t = numel / max(float(mk_t.sum()) * (C * V), 1.0)

    mv_eff = mk_s * f32(scale_s * scale_t)
    return mv_eff.astype(f32), mk_t.astype(f32)


def _build_bass(n_full, part):
    """Device program for one core: n_full [128,6400] fp16 tiles plus an
    optional [part,6400] tile; tile i multiplied in place by the
    per-partition scalar column sm[:, i] (s * mk_t, shipped as data),
    then stored.  Loads ride the SP HWDGE ring, stores the ACT HWDGE
    ring; every instruction carries at most one sync wait."""
    import concourse.bass as bass
    import concourse.mybir as mybir
    from concourse.tile import TileContext, add_dep_helper

    f16 = mybir.dt.float16
    ntiles = n_full + (1 if part else 0)
    rows = [P] * n_full + ([part] if part else [])
    rows_pc = n_full * P + (part or 0)

    nc = bass.Bass()
    xs = nc.dram_tensor("xs", [rows_pc, CV], f16, kind="ExternalInput")
    sm = nc.dram_tensor("sm", [P, ntiles], mybir.dt.float32,
                        kind="ExternalInput")
    out = nc.dram_tensor("out", [rows_pc, CV], f16, kind="ExternalOutput")

    with TileContext(nc) as tc:
        with tc.tile_pool(name="smt", bufs=1) as smtpool, \
             tc.tile_pool(name="scr", bufs=1) as scrpool, \
             tc.tile_pool(name="work", bufs=min(ntiles, 14)) as pool:
            ns = lambda a, b: add_dep_helper(a.ins, b.ins, sync=False,
                                             reason="tick ordering")
            smt = smtpool.tile([P, ntiles], mybir.dt.float32, tag="smt")
            # sm rides the SWDGE ring so the 8 HWDGE lane sems are left
            # for the (up to) 4+4 data loads/stores -- no lane reuse.
            ld_sm = nc.gpsimd.dma_start(smt[:, :], sm[:, :])
            # mask-carrier: the one DVE op that waits the sm load's lane
            # sem; every apply's own sm-lane wait is then elided and each
            # needs only its data load's wait.
            scr = scrpool.tile([P, 1], mybir.dt.float32, tag="scr")
            mcar = nc.vector.tensor_tensor(out=scr[:, :], in0=smt[:, 0:1],
                                           in1=smt[:, 0:1],
                                           op=mybir.AluOpType.mult)
            loads, stores, applies = [ld_sm], [], []
            for i in range(ntiles):
                r = rows[i]
                t = pool.tile([P, CV], f16)
                ld = nc.sync.dma_start(t[0:r, :], xs[i * P:i * P + r, :])

                ap = nc.vector.tensor_scalar_mul(
                    out=t[0:r, :], in0=t[0:r, :], scalar1=smt[0:r, i:i + 1])

                # store-ring lane absorber: an ACT no-op with a forced sync
                # dep on the load; it carries the load-lane wait so the
                # store (whose writer list still includes the load) needs
                # only its DVE wait.
                car = nc.scalar.nop()
                add_dep_helper(car.ins, ld.ins, sync=True,
                               reason="ring lane absorber")
                st = nc.scalar.dma_start(out[i * P:i * P + r, :], t[0:r, :])
                ns(st, car)
                ns(ap, mcar)
                if i >= 1:
                    ns(ap, applies[-1])          # keep DVE ticks monotone
                    ns(ld, loads[-1])
                    ns(st, stores[-1])
                loads.append(ld)
                stores.append(st)
                applies.append(ap)
            # Tail: absorb each outstanding sem into the SP sequencer's
            # observed set with a chain of 1-wait nops so the framework
            # drain never exceeds per-instruction wait capacity.
            prev = None
            tail_deps = list(stores) + list(loads) + [applies[-1]]
            for dep in tail_deps:
                nop = nc.sync.nop()
                add_dep_helper(nop.ins, dep.ins, sync=True,
                               reason="drain pre-absorb")
                add_dep_helper(nop.ins,
                               (prev if prev is not None else loads[-1]).ins,
                               sync=False, reason="tail order")
                prev = nop
    return nc


def kernel(x, A, mask_s, mask_t, u_s, u_t, w1, b1, bn_gamma, bn_beta,
           wh, bh, ww, bw):
    global LAST_RESULT
    from concourse.bass_utils import run_bass_kernel_spmd

    f16 = np.float16
    x = np.asarray(x, np.float32)
    mv_eff, mk_t = _compute_masks(A, mask_s, mask_t, u_s, u_t)

    surv = np.where((mv_eff != 0).any(axis=1) & (mk_t != 0).any(axis=1))[0]
    K = len(surv)
    out_full = np.zeros((NM, C, T, V), np.float32)
    if K == 0:
        return out_full

    scale = float(mv_eff[surv].max())         # the single surviving value
    rows_pc = K * T // N_CORES                # exact for any K
    n_full, part = divmod(rows_pc, P)
    ntiles = n_full + (1 if part else 0)

    # fp16 staging in [row=(n,t), col=(c,v)] layout; per-core inputs are
    # plain contiguous row-slices of it.
    xall = np.ascontiguousarray(
        x[surv].transpose(0, 2, 1, 3)).astype(f16).reshape(K * T, CV)
    # per-row mask scalar s*mk_t, arranged [core][partition, tile]
    smtall = (scale * mk_t[surv]).astype(np.float32).reshape(K * T)

    in_maps = []
    for k in range(N_CORES):
        rows_k = xall[k * rows_pc:(k + 1) * rows_pc]
        smk = np.zeros((ntiles * P,), np.float32)
        smk[:rows_pc] = smtall[k * rows_pc:(k + 1) * rows_pc]
        in_maps.append({
            "xs": rows_k,
            "sm": np.ascontiguousarray(smk.reshape(ntiles, P).T),
        })

    key = (n_full, part)
    if key not in _BASS:
        _BASS[key] = _build_bass(n_full, part)

    res = run_bass_kernel_spmd(_BASS[key], in_maps, list(range(N_CORES)),
                               trace=TRACE)
    LAST_RESULT = res

    outall = np.concatenate([res.results[k]["out"] for k in range(N_CORES)])
    out_full[surv] = outall.reshape(K, T, C, V).transpose(0, 2, 1, 3) \
        .astype(np.float32)
    # Measure-zero generality: a surviving batch with some (not all)
    # joints dropped gets those v columns zeroed exactly here.
    for j, n in enumerate(surv):
        dropped_v = np.flatnonzero(mv_eff[n] == 0)
        if len(dropped_v):
            out_full[n][:, :, dropped_v] = 0.0
    return out_full


# revision 12
# speedup vs baseline: 1.7698x; 1.1162x over previous
"""Fused DropBlock_Ske + DropBlockT_1d kernel for Trainium2 (8 NeuronCores).

The reference's coordinate-attention branch is dead code w.r.t. the output,
which reduces to

    out[n,c,t,v] = x[n,c,t,v] * mv_eff[n,v] * mk_t[n,t]

where mv_eff/mk_t are 0/s masks derived from the tiny inputs (mask_s,
mask_t, u_s, u_t, A).  Structural facts exploited:

  * DropBlock_Ske's adjacency propagation (M_seed @ A > eps) makes any
    batch with >=1 spatial seed (almost always) fully dropped, so a large
    fraction of batches is exactly zero and never touches the device.
  * Surviving batches keep all (or nearly all) joints, so the per-element
    multiplier is s * mk_t[n,t]: a function of (n,t) only.  Staged in a
    [t-partitions, (c,v)-cols] layout, it is a per-PARTITION scalar, so
    the whole masked multiply is one DVE tensor_scalar op per tile (4x
    perf mode) with the mask values shipped as data - which also keeps
    the single SPMD program valid for every core.
  * The rel-err budget (2e-2) is ~60x above fp16 staging error, so the
    surviving rows travel to/from HBM as fp16, halving DMA traffic.

Work splits at t-row granularity: K surviving batches = K*T rows of
C*V=6400 fp16 values, exactly K*16 rows per core for any K (for K=25:
3 full [128,6400] tiles + one [16,6400] partial).  Host work is only
sharding/staging: mask math on tiny inputs, batch selection, the
(n,c,t,v)->(n,t,c,v) transpose, fp16 conversion, and scattering device
results into the zeroed full-size output.  (In the measure-zero case of
a surviving batch with some joints dropped, those v-columns are zeroed
during the host-side scatter.)
"""

import numpy as np

NM, C, T, V = 64, 256, 128, 25
N_CORES = 8
CV = C * V                   # 6400
P = 128                      # SBUF partitions

KEEP_PROB = 0.9
BLOCK_SIZE = 7

# Set by test harness only: trace the run and stash results for profiling.
TRACE = False
LAST_RESULT = None

_BASS = {}


def _compute_masks(A, mask_s, mask_t, u_s, u_t):
    """Replicates the reference's mask math in float32 numpy.

    Returns mv_eff (NM, V) = mk_s * combined_scale and mk_t (NM, T)."""
    f32 = np.float32
    A = np.asarray(A, f32)
    mask_s = np.asarray(mask_s, f32)
    mask_t = np.asarray(mask_t, f32)
    u_s = np.asarray(u_s, f32)
    u_t = np.asarray(u_t, f32).reshape(NM, T)

    # ---- DropBlock_Ske ----
    gamma_s = f32((1.0 - KEEP_PROB) / (1.0 + 1.92))
    ms = mask_s / mask_s.sum() * f32(mask_s.size)
    p_s = np.minimum(ms * gamma_s, f32(1.0))
    m_seed = (u_s < p_s).astype(f32)
    m = ((m_seed @ A) > f32(0.001)).astype(f32)
    mk_s = f32(1.0) - m                                   # (NM, V), 0/1
    scale_s = float(NM * V) / max(float(mk_s.sum()), 1.0)

    # ---- DropBlockT_1d ----
    gamma_t = f32((1.0 - KEEP_PROB) / BLOCK_SIZE)
    mt = mask_t / mask_t.sum() * f32(mask_t.size)
    p_t = np.minimum(mt * gamma_t, 1.0)
    m_t = (u_t < p_t).astype(f32)                         # (NM, T), 0/1
    pad = BLOCK_SIZE // 2
    mp = np.pad(m_t, ((0, 0), (pad, pad)), constant_values=0.0)
    msum = m_t.copy()
    for i in range(BLOCK_SIZE):
        np.maximum(msum, mp[:, i:i + T], out=msum)
    mk_t = f32(1.0) - msum                                # (NM, T), 0/1
    numel = float(NM * C * T * V)
    scale_t = numel / max(float(mk_t.sum()) * (C * V), 1.0)

    mv_eff = mk_s * f32(scale_s * scale_t)
    return mv_eff.astype(f32), mk_t.astype(f32)


def _build_bass(n_full, part):
    """Device program for one core: n_full [128,6400] fp16 tiles plus an
    optional [part,6400] tile; tile i multiplied in place by the
    per-partition scalar column sm[:, i] (s * mk_t, shipped as data),
    then stored.  Loads ride the SP HWDGE ring, stores the ACT HWDGE
    ring; every instruction carries at most one sync wait."""
    import concourse.bass as bass
    import concourse.mybir as mybir
    from concourse.tile import TileContext, add_dep_helper

    f16 = mybir.dt.float16
    ntiles = n_full + (1 if part else 0)
    rows = [P] * n_full + ([part] if part else [])
    rows_pc = n_full * P + (part or 0)

    nc = bass.Bass()
    xs = nc.dram_tensor("xs", [rows_pc, CV], f16, kind="ExternalInput")
    sm = nc.dram_tensor("sm", [P, ntiles], mybir.dt.float32,
                        kind="ExternalInput")
    out = nc.dram_tensor("out", [rows_pc, CV], f16, kind="ExternalOutput")

    with TileContext(nc) as tc:
        with tc.tile_pool(name="smt", bufs=1) as smtpool, \
             tc.tile_pool(name="scr", bufs=1) as scrpool, \
             tc.tile_pool(name="pscr", bufs=ntiles) as ppool, \
             tc.tile_pool(name="work", bufs=min(ntiles, 14)) as pool:
            ns = lambda a, b: add_dep_helper(a.ins, b.ins, sync=False,
                                             reason="tick ordering")
            smt = smtpool.tile([P, ntiles], mybir.dt.float32, tag="smt")
            # sm rides the SWDGE ring so the 8 HWDGE lane sems are left
            # for the (up to) 4+4 data loads/stores -- no lane reuse.
            ld_sm = nc.gpsimd.dma_start(smt[:, :], sm[:, :])
            # mask-carrier: the one DVE op that waits the sm load's lane
            # sem; every apply's own sm-lane wait is then elided and each
            # needs only its data load's wait.
            scr = scrpool.tile([P, 1], mybir.dt.float32, tag="scr")
            mcar = nc.vector.tensor_tensor(out=scr[:, :], in0=smt[:, 0:1],
                                           in1=smt[:, 0:1],
                                           op=mybir.AluOpType.mult)
            loads, stores, applies = [ld_sm], [], []
            for i in range(ntiles):
                r = rows[i]
                t = pool.tile([P, CV], f16)
                ld = nc.sync.dma_start(t[0:r, :], xs[i * P:i * P + r, :])

                ap = nc.vector.tensor_scalar_mul(
                    out=t[0:r, :], in0=t[0:r, :], scalar1=smt[0:r, i:i + 1])

                # Stores ride the SWDGE (row-0) ring: the SDMA engines'
                # strict row priority preempts the HWDGE load row per
                # packet, so stores overlap loads instead of waiting for
                # the whole load queue to drain.  pcar is a write-only
                # no-op with a forced sync dep on the load; it carries the
                # load-lane wait so the store (whose writer list still
                # includes the load) needs only its DVE wait.
                pscr = ppool.tile([1, 1], mybir.dt.float32)
                car = nc.gpsimd.memset(pscr[0:1, 0:1], 0.0)
                add_dep_helper(car.ins, ld.ins, sync=True,
                               reason="ring lane absorber")
                st = nc.gpsimd.dma_start(out[i * P:i * P + r, :], t[0:r, :])
                ns(st, car)
                ns(ap, mcar)
                if i >= 1:
                    ns(ap, applies[-1])          # keep DVE ticks monotone
                    ns(ld, loads[-1])
                    ns(st, stores[-1])
                loads.append(ld)
                stores.append(st)
                applies.append(ap)
            # Tail: absorb each outstanding sem into the SP sequencer's
            # observed set with a chain of 1-wait nops so the framework
            # drain never exceeds per-instruction wait capacity.
            prev = None
            tail_deps = list(stores) + list(loads) + [applies[-1]]
            for dep in tail_deps:
                nop = nc.sync.nop()
                add_dep_helper(nop.ins, dep.ins, sync=True,
                               reason="drain pre-absorb")
                add_dep_helper(nop.ins,
                               (prev if prev is not None else loads[-1]).ins,
                               sync=False, reason="tail order")
                prev = nop
    return nc


def kernel(x, A, mask_s, mask_t, u_s, u_t, w1, b1, bn_gamma, bn_beta,
           wh, bh, ww, bw):
    global LAST_RESULT
    from concourse.bass_utils import run_bass_kernel_spmd

    f16 = np.float16
    x = np.asarray(x, np.float32)
    mv_eff, mk_t = _compute_masks(A, mask_s, mask_t, u_s, u_t)

    surv = np.where((mv_eff != 0).any(axis=1) & (mk_t != 0).any(axis=1))[0]
    K = len(surv)
    out_full = np.zeros((NM, C, T, V), np.float32)
    if K == 0:
        return out_full

    scale = float(mv_eff[surv].max())         # the single surviving value
    rows_pc = K * T // N_CORES                # exact for any K
    n_full, part = divmod(rows_pc, P)
    ntiles = n_full + (1 if part else 0)

    # fp16 staging in [row=(n,t), col=(c,v)] layout; per-core inputs are
    # plain contiguous row-slices of it.
    xall = np.ascontiguousarray(
        x[surv].transpose(0, 2, 1, 3)).astype(f16).reshape(K * T, CV)
    # per-row mask scalar s*mk_t, arranged [core][partition, tile]
    smtall = (scale * mk_t[surv]).astype(np.float32).reshape(K * T)

    in_maps = []
    for k in range(N_CORES):
        rows_k = xall[k * rows_pc:(k + 1) * rows_pc]
        smk = np.zeros((ntiles * P,), np.float32)
        smk[:rows_pc] = smtall[k * rows_pc:(k + 1) * rows_pc]
        in_maps.append({
            "xs": rows_k,
            "sm": np.ascontiguousarray(smk.reshape(ntiles, P).T),
        })

    key = (n_full, part)
    if key not in _BASS:
        _BASS[key] = _build_bass(n_full, part)

    res = run_bass_kernel_spmd(_BASS[key], in_maps, list(range(N_CORES)),
                               trace=TRACE)
    LAST_RESULT = res

    outall = np.concatenate([res.results[k]["out"] for k in range(N_CORES)])
    out_full[surv] = outall.reshape(K, T, C, V).transpose(0, 2, 1, 3) \
        .astype(np.float32)
    # Measure-zero generality: a surviving batch with some (not all)
    # joints dropped gets those v columns zeroed exactly here.
    for j, n in enumerate(surv):
        dropped_v = np.flatnonzero(mv_eff[n] == 0)
        if len(dropped_v):
            out_full[n][:, :, dropped_v] = 0.0
    return out_full
